# revision 85
# baseline (speedup 1.0000x reference)
"""Deformable Conv2d (3x3, pad=1, stride=1) on Trainium2 — Bass/Tile kernel.

Sharding: data-parallel over batch across 8 NeuronCores (B=8 -> 1 image/core);
weights replicated. Per-core pipeline (all 16-bit work in fp16):
  host prep: padded fp16 channel-major image (66-wide grid) for the offset
             conv; a DRAM gather source xt3[i] = [x(i-65) | x(i-1)] so ONE
             2KB gather elem (rows i, i+1, elem_step=512) yields all four
             bilinear corners of a (pixel, tap) sample; fp16 weight packs.
  per-chunk prep (1024 pixels, pipelined ahead of the main loop):
    phase A: offset conv (18ch 3x3) as PSUM-accumulated PE matmuls with
             contiguous rhs windows over the padded-66 grid.
    phase B: bilinear corner weights + single per-(pixel,tap) gather index
             on DVE in a pixel-major layout (partition = pixel%128);
             floor() via the 1.5*2^23 magic-add trick; OOB corners get zero
             weight (matches the reference's zero-pad semantics).
  per-chunk main loop, per tap: one SWDGE dma_gather (1024 idx x 2KB,
    prefetched one tap ahead, alternating 2 SWDGE queues) ->
    g[128, 8, 1024] holding corners (y0x0|y1x0|y0x1|y1x1) per j-group;
    corner weighting as 4 DVE tensor_scalar ops (4x fp16 mode, per-pixel
    partition scalars); one pair-add on Pool/DVE; the remaining corner sum
    is folded into the channel-major transposes, done as regular fp16
    matmuls against identity that accumulate in fp32 PSUM; PSUM->SBUF
    copies on ACT; main conv as PSUM-accumulated fp16 matmuls (36
    accumulation steps); fp16 output staged ACT/DVE and DMA'd out.
"""
import sys

sys.path.insert(0, "/opt/trn_rl_repo")

import numpy as np
import ml_dtypes

import concourse.mybir as mybir
from concourse import bacc
from concourse import bass_utils
from concourse.tile import TileContext
from concourse.bass_types import AP
from concourse.masks import make_identity

B, C, O, H, W = 8, 256, 256, 64, 64
HW = H * W                  # 4096
NCORES = 8
NCHUNK = 4                  # pixel chunks in the main loop
CH = HW // NCHUNK           # 1024 pixels / chunk
JG = CH // 128              # 8 j-groups of 128 pixels / chunk
CR = H // NCHUNK            # 16 image rows / chunk
W2 = W + 2                  # padded row width (66)
XROWS = 4232                # xt3 rows: 65 lead guard + 4096 + 71 tail
MAGIC = 12582912.0          # 1.5 * 2^23: float32 round-to-int bias
AluOp = mybir.AluOpType


def _emit(nc):
    f32, f16, i16 = mybir.dt.float32, mybir.dt.float16, mybir.dt.int16

    x_pad_d = nc.dram_tensor("xpad", [128, 4 * 2 * 19 * W2], f16,
                             kind="ExternalInput")
    offw = nc.dram_tensor("offw", [128, 2, 9, 18], f16, kind="ExternalInput")
    offb = nc.dram_tensor("offb", [18, 1], f32, kind="ExternalInput")
    convw = nc.dram_tensor("convw", [128, 18, 256], f16, kind="ExternalInput")
    kgrid_d = nc.dram_tensor("kgrid", [128, 32, 18], f32, kind="ExternalInput")
    xt3 = nc.dram_tensor("xt3", [XROWS, 512], f16, kind="ExternalInput")
    y_out = nc.dram_tensor("y", [O, H * W], f16, kind="ExternalOutput")

    with TileContext(nc) as tc:
        with tc.tile_pool(name="consts", bufs=1) as consts, \
             tc.tile_pool(name="pb", bufs=1) as pb, \
             tc.tile_pool(name="gather", bufs=3) as gp, \
             tc.tile_pool(name="ps_a", bufs=2, space="PSUM") as ps_a, \
             tc.tile_pool(name="ps_tp", bufs=2, space="PSUM") as ps_tp, \
             tc.tile_pool(name="ps_acc", bufs=1, space="PSUM") as ps_acc:
            # PSUM banks: accs 4 + stp 2 + pa/offt (shared tag) 2 = 8

            # offset-conv weights first: they gate phase_a(0)'s ldweights
            offw_sb = consts.tile([128, 2, 9, 18], f16)
            nc.sync.dma_start(out=offw_sb[:], in_=offw.ap())

            # ---- padded fp16 image as per-chunk slabs (18 grid rows
            # each, 2-row overlap baked on host) so phase_a(c) depends
            # only on its own 0.6MB slab ----
            SLAB = 2 * 19 * W2
            x_slabs = []
            for c in range(4):
                xs_t = pb.tile([128, 2, 19, W2], f16, name=f"xslab_{c}",
                               tag=f"xslab_{c}")
                nc.sync.dma_start(
                    out=xs_t.rearrange("c a b w -> c (a b w)"),
                    in_=x_pad_d.ap()[:, c * SLAB:(c + 1) * SLAB])
                x_slabs.append(xs_t.rearrange("c cc h w -> c cc (h w)"))

            # ---- constants / weights to SBUF ----
            ident = consts.tile([128, 128], f16)
            make_identity(nc, ident)
            ident_f32 = consts.tile([128, 128], f32)
            make_identity(nc, ident_f32)
            # PE p-state warmup: ~4us of dummy transposes while inputs load,
            # so phase_a runs at full clock
            for wu in range(24):
                pwu = ps_tp.tile([128, 128], f16, tag="stp", name="warm")
                nc.tensor.transpose(pwu[:], ident[:], ident[:])
            offb_sb = consts.tile([18, 1], f32)
            nc.sync.dma_start(out=offb_sb[:], in_=offb.ap())
            kgrid = consts.tile([128, 32, 18], f32)
            nc.sync.dma_start(out=kgrid[:], in_=kgrid_d.ap())
            convw_sb = consts.tile([128, 18, 256], f16)

            # ================= per-chunk prep =================
            def phase_a(c):
                """Offset conv for image rows [16c, 16c+16) -> off66_c."""
                off66 = pb.tile([18, CR * W2], f32, name=f"off66_{c}",
                                tag=f"off66_{c}")
                x_flat_c = x_slabs[c]
                for t, (r0, rows) in enumerate(((0, 7), (7, 7), (14, 2))):
                    n = rows * W2
                    pa = ps_a.tile([18, 462], f32, tag="pa", name="pa")
                    for k in range(9):
                        ky, kx = k // 3, k % 3
                        base = (r0 + ky) * W2 + kx
                        for cc in range(2):
                            nc.tensor.matmul(
                                pa[:, 0:n],
                                offw_sb[:, cc, k, :],
                                x_flat_c[:, cc, base:base + n],
                                start=(k == 0 and cc == 0),
                                stop=(k == 8 and cc == 1))
                    nc.vector.tensor_scalar(
                        out=off66[:, r0 * W2:r0 * W2 + n], in0=pa[:, 0:n],
                        scalar1=offb_sb[:, 0:1], scalar2=None, op0=AluOp.add)
                return off66

            def phase_b(c, off66):
                """Corner weights + single gather index per (pixel, tap)."""
                # pixel-major offsets: offpx[q, jl, ch] (p = (c*8+jl)*128+q)
                offpx = pb.tile([128, JG, 18], f32, name=f"offpx_{c}",
                                tag=f"offpx_{c}")
                for hl in range(CR):
                    pt = ps_a.tile([64, 18], f32, tag="pa", name="offt")
                    nc.tensor.transpose(
                        pt[:], off66[:, hl * W2:hl * W2 + W],
                        ident_f32[0:18, 0:18])
                    nc.scalar.copy(
                        offpx[(hl % 2) * 64:(hl % 2) * 64 + 64, hl // 2, :],
                        pt[:])

                shp = [128, JG, 18]
                tl = {n: pb.tile(shp, f32, name=f"{n}_{c}", tag=n, bufs=2)
                      for n in ("PP", "FF", "II", "M0", "M1", "U0", "U1",
                                "T1")}
                shp9 = [128, JG, 9]
                cyx = {n: pb.tile(shp9, f32, name=f"{n}_{c}", tag=n, bufs=2)
                       for n in ("CY", "CX")}
                w4 = pb.tile([128, 9, 4, JG], f32, name=f"w4_{c}", tag=f"w4_{c}")
                tb = pb.tile([128, JG, 9], f32, name=f"tb_{c}", tag="tb",
                             bufs=2)

                def ts(out, in0, s, op, s2=None, op2=None):
                    if op2 is None:
                        nc.vector.tensor_scalar(out=out, in0=in0, scalar1=s,
                                                scalar2=None, op0=op)
                    else:
                        nc.vector.tensor_scalar(out=out, in0=in0, scalar1=s,
                                                scalar2=s2, op0=op, op1=op2)

                PP, FF, II = tl["PP"], tl["FF"], tl["II"]
                M0, M1, U0, U1, T1 = (tl["M0"], tl["M1"], tl["U0"], tl["U1"],
                                      tl["T1"])
                CY, CX = cyx["CY"], cyx["CX"]
                nc.vector.tensor_add(PP[:], offpx[:],
                                     kgrid[:, c * JG:(c + 1) * JG, :])
                ts(T1[:], PP[:], 0.5, AluOp.subtract, MAGIC, AluOp.add)
                ts(II[:], T1[:], MAGIC, AluOp.subtract)    # II = floor(PP)
                nc.vector.tensor_sub(FF[:], PP[:], II[:])  # frac in [0,1)
                ts(M0[:], II[:], 0.0, AluOp.is_ge)
                ts(T1[:], II[:], 63.0, AluOp.is_le)
                nc.vector.tensor_mul(M0[:], M0[:], T1[:])
                ts(M1[:], II[:], -1.0, AluOp.is_ge)
                ts(T1[:], II[:], 62.0, AluOp.is_le)
                nc.vector.tensor_mul(M1[:], M1[:], T1[:])
                nc.vector.tensor_mul(T1[:], FF[:], M0[:])
                nc.vector.tensor_sub(U0[:], M0[:], T1[:])  # (1-f)*m0
                nc.vector.tensor_mul(U1[:], FF[:], M1[:])  # f*m1
                # corner blocks in the gathered elem: (y0x0, y1x0, y0x1, y1x1)
                Us = (U0, U1)
                for xs in range(2):
                    for i in range(2):
                        nc.gpsimd.tensor_mul(
                            w4[:, :, xs * 2 + i, :].rearrange("p k j -> p j k"),
                            Us[i][:, :, 0:18:2], Us[xs][:, :, 1:18:2])
                # single gather index: R = (cy+1)*64 + cx + 1 with
                # cy = clamp(floor_y, -1, 63), cx = clamp(floor_x, -1, 63).
                ts(CY[:], II[:, :, 0:18:2], 63.0, AluOp.min, -1.0, AluOp.max)
                ts(CX[:], II[:, :, 1:18:2], 63.0, AluOp.min, -1.0, AluOp.max)
                nc.vector.scalar_tensor_tensor(
                    out=tb[:], in0=CY[:], scalar=64.0, in1=CX[:],
                    op0=AluOp.mult, op1=AluOp.add)
                ts(tb[:], tb[:], 65.0, AluOp.add)

                # wrap to (p%16, p//16) idx layout via exact f32 PE
                # transposes (values <= 4160, exactly representable)
                idxw = pb.tile([128, 9, CH // 16], i16, name=f"idxw_{c}",
                               tag=f"idxw_{c}")
                ptb = ps_a.tile([72, 128], f32, tag="pa", name="ptb")
                nc.tensor.transpose(
                    ptb[:], tb.rearrange("p k c -> p (k c)"), ident_f32[:])
                tbT = pb.tile([72, 128], f32, name=f"tbT_{c}", tag="tbT",
                              bufs=2)
                nc.vector.tensor_copy(tbT[:], ptb[:])
                for qh in range(8):
                    pq = ps_a.tile([16, 72], f32, tag="pa", name="pq")
                    nc.tensor.transpose(pq[:], tbT[:, qh * 16:(qh + 1) * 16],
                                        ident_f32[0:72, 0:72])
                    cp_out = idxw[0:16, :, qh:CH // 16:8]
                    cp_in = pq.rearrange("r (k c) -> r c k", k=JG)
                    if qh % 2 == 0:
                        nc.scalar.copy(cp_out, cp_in)
                    else:
                        nc.vector.tensor_copy(cp_out, cp_in)
                # replicate to all 8 16-partition groups (parallel DMAs)
                for m0 in range(1, 8):
                    nc.sync.dma_start(out=idxw[m0 * 16:(m0 + 1) * 16, :, :],
                                      in_=idxw[0:16, :, :])
                return w4, idxw

            # chunk-0 prep first (highest priority: first gathers gate all)
            prep = {}
            prep[0] = phase_b(0, phase_a(0))

            # ================= main loop =================
            xt_win = AP(tensor=xt3, offset=0, ap=[[512, XROWS - 1], [1, 1024]])
            gtiles = {}

            def issue_gather(c, k):
                if (c, k) in gtiles:
                    return
                _, idxw_c = prep[c]
                g = gp.tile([128, JG, 1024], f16, tag="g", name="g",
                            bufs=4)
                nc.gpsimd.dma_gather(
                    out_ap=g[:], in_ap=xt_win,
                    idxs_ap=idxw_c[:, k, :],
                    num_idxs=CH, num_idxs_reg=CH,
                    elem_size=1024, elem_step=512,
                    transpose=False,
                    queue_num=(c * 9 + k) % 2)
                gtiles[(c, k)] = g

            issue_gather(0, 0)
            prep[1] = phase_b(1, phase_a(1))
            # conv weights are first needed ~50us in; load after the
            # startup-critical DMAs
            nc.sync.dma_start(out=convw_sb[:], in_=convw.ap())
            for ch in range(NCHUNK):
                # prep runs two chunks ahead of the consuming gathers
                if ch + 2 < NCHUNK:
                    prep[ch + 2] = phase_b(ch + 2, phase_a(ch + 2))
                w4, idxw = prep[ch]
                accs = [ps_acc.tile([128, 512], f32, tag=f"acc{a}",
                                    name=f"acc{a}") for a in range(4)]
                for k in range(9):
                    # prefetch the next tap's gather ahead of this combine
                    if k + 1 < 9:
                        issue_gather(ch, k + 1)
                    elif ch + 1 < NCHUNK:
                        issue_gather(ch + 1, 0)
                    g = gtiles.pop((ch, k))
                    s_t = gp.tile([128, JG, 256], f16, tag="s", name="s",
                                  bufs=3)
                    sk = gp.tile([128, 2, CH], f16, tag="sk", name="sk",
                                 bufs=3)
                    for j in range(JG):
                        # weighted corners: corner 0 on ACT (scale-ptr copy),
                        # corners 1-3 on DVE tensor_scalar (4x fp16 mode);
                        # one pair-add on DVE, remaining sum folded into
                        # PSUM-accumulating transposes on PE.
                        mt = gp.tile([128, 4, 256], f16, tag="mt", name="mt",
                                     bufs=4)
                        for corner in (0, 1, 2, 3):
                            nc.vector.tensor_scalar(
                                out=mt[:, corner, :],
                                in0=g[:, j, corner * 256:(corner + 1) * 256],
                                scalar1=w4[:, k, corner, j:j + 1],
                                scalar2=None, op0=AluOp.mult)
                        sc = gp.tile([128, 256], f16, tag="sc", name="sc",
                                     bufs=4)
                        add_eng = nc.gpsimd if j % 8 < 6 else nc.vector
                        add_eng.tensor_add(sc[:], mt[:, 0, :], mt[:, 1, :])
                        # regular fp16 matmuls against identity = transposes
                        # that genuinely accumulate in fp32 PSUM
                        ptp = ps_tp.tile([128, 256], f32, tag="stp",
                                         name="stp")
                        for cc in range(2):
                            h = slice(cc * 128, (cc + 1) * 128)
                            o = ptp[:, h]
                            # sc (the Pool add) last, so the first two
                            # transposes don't wait on Pool
                            nc.tensor.matmul(o, mt[:, 2, h], ident[:],
                                             start=True, stop=False)
                            nc.tensor.matmul(o, mt[:, 3, h], ident[:],
                                             start=False, stop=False)
                            nc.tensor.matmul(o, sc[:, h], ident[:],
                                             start=False, stop=True)
                        # PSUM->SBUF copy (both cc halves): on ACT
                        cp_out = sk[:, :, j * 128:(j + 1) * 128]
                        cp_in = ptp.rearrange("p (c x) -> p c x", c=2)
                        nc.scalar.copy(cp_out, cp_in)
                    for cc in range(2):
                        for o in range(2):
                            for sub in range(2):
                                nc.tensor.matmul(
                                    accs[o * 2 + sub],
                                    convw_sb[:, k * 2 + cc,
                                             o * 128:(o + 1) * 128],
                                    sk[:, cc, sub * 512:(sub + 1) * 512],
                                    start=(k == 0 and cc == 0),
                                    stop=(k == 8 and cc == 1))
                for o in range(2):
                    ob = gp.tile([128, CH], f16, tag=f"ob{o}",
                                 name=f"ob{o}", bufs=2)
                    for sub in range(2):
                        if o == 0:
                            nc.scalar.copy(ob[:, sub * 512:(sub + 1) * 512],
                                           accs[o * 2 + sub][:])
                        else:
                            nc.vector.tensor_copy(
                                ob[:, sub * 512:(sub + 1) * 512],
                                accs[o * 2 + sub][:])
                    nc.sync.dma_start(
                        out=y_out.ap()[o * 128:(o + 1) * 128,
                                       ch * CH:(ch + 1) * CH],
                        in_=ob[:])
    nc.compile()
    return nc


_CACHE = {}


def _get_nc():
    if "nc" not in _CACHE:
        nc = bacc.Bacc("TRN2", target_bir_lowering=False, debug=False,
                       num_devices=NCORES,
                       dynamic_dma_scratch_size=32768,
                       num_swdge_queues=2)
        _CACHE["nc"] = _emit(nc)
    return _CACHE["nc"]


def _host_tables():
    if "kgrid" in _CACHE:
        return _CACHE["kgrid"]
    q = np.arange(128)[:, None, None]
    j = np.arange(32)[None, :, None]
    c = np.arange(18)[None, None, :]
    p = j * 128 + q
    k = c // 2
    d = c % 2
    ky, kx = k // 3, k % 3
    grid = np.where(d == 0, p // W + ky - 1, p % W + kx - 1).astype(np.float32)
    _CACHE["kgrid"] = np.ascontiguousarray(grid)
    return _CACHE["kgrid"]


def _pack_weights(offset_w, offset_b, conv_w):
    # offw lhsT: [c, cc, k, j] = offset_w[j, cc*128+c, ky, kx]
    ow = offset_w.reshape(18, 2, 128, 9).transpose(2, 1, 3, 0)
    # convw lhsT: [c, (k,cc) chunk, o] = conv_w[o, cc*128+c, k]
    cw = conv_w.reshape(256, 2, 128, 9).transpose(2, 3, 1, 0)  # c, k, cc, o
    cw = cw.reshape(128, 18, 256)
    ob = offset_b.reshape(18, 1)
    return (np.ascontiguousarray(ow, np.float16),
            np.ascontiguousarray(ob, np.float32),
            np.ascontiguousarray(cw, np.float16))


def make_in_maps(x, offset_w, offset_b, conv_w):
    ow, ob, cw = _pack_weights(np.asarray(offset_w), np.asarray(offset_b),
                               np.asarray(conv_w))
    kg = _host_tables()
    x16 = np.asarray(x).astype(np.float16)  # [B, 256, 64, 64]
    in_maps = []
    for b in range(B):
        xc = x16[b].reshape(C, HW)
        # padded channel-major image [128, 2, 67, 66] -> 4 slabs of 18
        # grid rows (2-row overlap) so each chunk's offset conv can start
        # as soon as its own slab lands
        xp_full = np.zeros((128, 2, H + 3, W2), dtype=np.float16)
        xp_full[:, :, 1:H + 1, 1:W + 1] = (
            xc.reshape(2, 128, H, W).transpose(1, 0, 2, 3))
        xp = np.stack([xp_full[:, :, 16 * c:16 * c + 19, :]
                       for c in range(4)], axis=1)
        # gather source: xt3[i] = [x(i-65) | x(i-1)] pixel-major
        xf = np.ascontiguousarray(xc.T)  # [4096, 256]
        xt = np.zeros((XROWS, 512), dtype=np.float16)
        xt[65:65 + HW, 0:256] = xf
        xt[1:1 + HW, 256:512] = xf
        in_maps.append({
            "xpad": np.ascontiguousarray(xp.reshape(128, -1)),
            "offw": ow, "offb": ob, "convw": cw, "kgrid": kg,
            "xt3": xt})
    return in_maps


def kernel(x, offset_w, offset_b, conv_w):
    nc = _get_nc()
    in_maps = make_in_maps(x, offset_w, offset_b, conv_w)
    res = bass_utils.run_bass_kernel_spmd(nc, in_maps,
                                          core_ids=list(range(NCORES)))
    out = np.stack([np.asarray(res.results[b]["y"], dtype=np.float32)
                    .reshape(O, H, W) for b in range(B)])
    return out
